# revision 31
# baseline (speedup 1.0000x reference)
"""HCNN known-U cell on 8 Trainium2 NeuronCores (Bass/Tile).

Math (from the reference, simplified):
    output    = state[:, :16] - observation
    pre       = [observation, state[:, 16:], known_features]     # (B, 288)
    new_state = tanh(pre) @ A_weight.T                           # (B, 256)

Strategy: data-parallel over the batch (8192 rows/core). The host hands each
core *feature-major* (transposed) shards so the contraction dim (288 input
features) lands on SBUF partitions with no on-chip transposes, and every DMA
is a clean 2D strided copy with multi-KB lines. A_weight is replicated,
pre-transposed to (288, 256) so K-chunks of A^T serve as stationary lhsT.

The 288 contraction features are processed in a permuted order so that every
SBUF partition access starts 32-aligned (HW requirement):
    chunk0 (K=128): state feats  16:144
    chunk1 (K=112): state feats 144:256
    chunk2 (K= 48): known feats (32) then obs feats (16)  [obs at base 32]
A^T rows are permuted identically on the host; the K-sum is order-invariant.

Per core:
  inputs : state_t (256,8192), known_t (32,8192), obs_t (16,8192),
           a_t (288,256) permuted as above
  outputs: new_state_t (256,8192), out_t (16,8192)
"""

import sys

sys.path.insert(0, "/opt/trn_rl_repo")

import numpy as np

import concourse.bacc as bacc
import concourse.bass as bass
import concourse.tile as tile
from concourse import mybir
from concourse.bass_utils import run_bass_kernel_spmd

B, S, U, Y = 65536, 256, 32, 16
K = S + U  # 288 contraction features
M = 8  # cores
R = B // M  # 8192 batch rows per core
F = 1024  # batch columns per group
NSUB = 512  # matmul moving free dim (one PSUM bank of fp32)
FP = mybir.dt.float32
MM_DT = mybir.dt.float16  # matmul operand dtype: fp16 = full rate + FWL, ~3e-4 rel err

_nc_cache = {}


def _build(mm_dt):
    # Bacc (not raw Bass): its compile() pipeline splits multi-wait sync
    # conditions into event-semaphore instructions (TRN2 allows only one
    # sync wait per instruction).
    nc = bacc.Bacc("TRN2", target_bir_lowering=False, debug=False, num_devices=M)
    # state cols 16:256 (the only ones tanh needs) travel as fp16; cols 0:16
    # (only used for the exact output subtract) travel as fp32 in s0_t.
    st = nc.declare_dram_parameter("sm_t", [S - Y, R], mm_dt, isOutput=False)
    ko = nc.declare_dram_parameter("ko_t", [U + Y, R], FP, isOutput=False)
    s0t = nc.declare_dram_parameter("s0_t", [Y, R], FP, isOutput=False)
    at = nc.declare_dram_parameter("a_t", [128, 3 * S], mm_dt, isOutput=False)
    ns = nc.declare_dram_parameter("new_state_t", [S, R], mm_dt, isOutput=True)
    o2 = nc.declare_dram_parameter("out_t", [Y, R], FP, isOutput=True)

    Tanh = mybir.ActivationFunctionType.Tanh

    with tile.TileContext(nc) as tc:
        with (
            tc.tile_pool(name="const", bufs=1) as cpool,
            tc.tile_pool(name="ins", bufs=8) as ipool,
            tc.tile_pool(name="acts", bufs=6) as apool,
            tc.tile_pool(name="outs", bufs=6) as opool,
            tc.tile_pool(name="ps", bufs=3, space="PSUM") as pspool,
        ):
            # A^T packed on host into one (128, 768) block: cols 0:256 are
            # K-chunk0 (128 rows), 256:512 chunk1 (112 rows + pad), 512:768
            # chunk2 (48 rows + pad) -> a single DMA, fewer sync waits on the
            # first matmul.
            apk = cpool.tile([128, 3 * S], mm_dt, tag="apk")
            nc.sync.dma_start(apk[:], at[:, :])
            a0 = apk[:, 0:S]
            a1 = apk[0:112, S : 2 * S]
            a2 = apk[0:48, 2 * S : 3 * S]

            # Warm-up: ~20 dense dummy matmuls while the first DMAs stream,
            # so the PE HAM un-throttles (1.2 -> 2.4 GHz) before real work.
            wl = cpool.tile([128, 128], mm_dt, tag="wl")
            wr = cpool.tile([128, NSUB], mm_dt, tag="wr")
            nc.vector.memset(wl[:], 0.0)
            nc.vector.memset(wr[:], 0.0)
            wps = pspool.tile([128, NSUB], FP, tag="wps", name="wps", bufs=1)
            for _ in range(10):
                nc.tensor.matmul(wps[:], wl[:], wr[:], start=True, stop=True)

            for j in range(0, R, F):
                js = slice(j, j + F)
                # state feats 16:144 / 144:256 (fp16); known+obs packed fp32
                p0 = ipool.tile([128, F], mm_dt, tag="p0")
                p1 = ipool.tile([112, F], mm_dt, tag="p1")
                p2 = ipool.tile([48, F], FP, tag="p2")
                # state[:, :16] parked at base partition 32 so the subtract's
                # operands share a base partition (HW TensorTensor rule).
                s0 = ipool.tile([48, F], FP, tag="s0")
                nc.sync.dma_start(p0[:], st[0:128, js])
                nc.sync.dma_start(p1[:], st[128:240, js])
                nc.sync.dma_start(p2[:], ko[:, js])
                nc.sync.dma_start(s0[U : U + Y, :], s0t[:, js])

                # output = state[:, :16] - observation  (transposed layout)
                o2t = opool.tile([48, F], FP, tag="o2")
                nc.vector.tensor_sub(
                    o2t[U : U + Y, :], s0[U : U + Y, :], p2[U : U + Y, :]
                )
                nc.gpsimd.dma_start(o2[:, js], o2t[U : U + Y, :])

                # rstate^T = tanh(pre^T) with K features on partitions
                t0 = apool.tile([128, F], mm_dt, tag="t0")
                t1 = apool.tile([112, F], mm_dt, tag="t1")
                t2 = apool.tile([48, F], mm_dt, tag="t2")
                nc.scalar.activation(t0[:], p0[:], Tanh)
                nc.scalar.activation(t1[:], p1[:], Tanh)
                nc.scalar.activation(t2[:], p2[:], Tanh)

                # 2 filler matmuls per group: keep the PE HAM activity window
                # non-idle so the clock stays at 2.4 GHz through DMA stalls.
                nc.tensor.matmul(wps[0:64, 0:64], wl[:, 0:64], wr[:, 0:64], start=True, stop=True)

                n0 = opool.tile([128, F], mm_dt, tag="n0")
                n1 = opool.tile([128, F], mm_dt, tag="n1")
                # k-outer within each output half: the same 128x128 weight
                # block feeds all N-chunks back-to-back (one weight load per
                # chunk instead of one per matmul).
                for lo, ntile in ((0, n0), (128, n1)):
                    ps = pspool.tile([128, F], FP, tag="ps", name="ps")
                    for ki, (ak, tk) in enumerate(((a0, t0), (a1, t1), (a2, t2))):
                        for c in range(0, F, NSUB):
                            nc.tensor.matmul(
                                ps[:, c : c + NSUB],
                                ak[:, lo : lo + 128],
                                tk[:, c : c + NSUB],
                                start=(ki == 0),
                                stop=(ki == 2),
                            )
                    nc.vector.tensor_copy(ntile[:], ps[:])
                    nc.tensor.matmul(wps[0:64, 0:64], wl[:, 0:64], wr[:, 0:64], start=True, stop=True)
                nc.gpsimd.dma_start(ns[0:128, js], n0[:])
                nc.sync.dma_start(ns[128:256, js], n1[:])
    nc.finalize()
    return nc


def get_nc():
    key = str(MM_DT)
    if key not in _nc_cache:
        _nc_cache[key] = _build(MM_DT)
    return _nc_cache[key]


def _np_mm_dtype():
    if MM_DT == mybir.dt.bfloat16:
        import ml_dtypes

        return ml_dtypes.bfloat16
    if MM_DT == mybir.dt.float16:
        return np.float16
    return np.float32


def make_in_maps(state, known, obs, A):
    at = np.ascontiguousarray(A.T)  # (288, 256); row i = input feature i
    # Pack permuted K-chunks side by side into one (128, 768) block (see
    # module docstring); pad rows are zero and multiply into nothing.
    a_perm = np.zeros((128, 3 * S), dtype=np.float32)
    a_perm[0:128, 0:S] = at[16:144]
    a_perm[0:112, S : 2 * S] = at[144:256]
    a_perm[0:32, 2 * S : 3 * S] = at[256:288]
    a_perm[32:48, 2 * S : 3 * S] = at[0:16]
    a_perm = a_perm.astype(_np_mm_dtype())
    in_maps = []
    for i in range(M):
        sl = slice(i * R, (i + 1) * R)
        in_maps.append(
            {
                "sm_t": np.ascontiguousarray(state[sl, Y:].T).astype(_np_mm_dtype()),
                "ko_t": np.ascontiguousarray(
                    np.concatenate([known[sl], obs[sl]], axis=1).T
                ),
                "s0_t": np.ascontiguousarray(state[sl, :Y].T),
                "a_t": a_perm,
            }
        )
    return in_maps


def assemble(results):
    new_state = np.empty((B, S), np.float32)
    output = np.empty((B, Y), np.float32)
    for i in range(M):
        sl = slice(i * R, (i + 1) * R)
        new_state[sl] = results[i]["new_state_t"].T.astype(np.float32)
        output[sl] = results[i]["out_t"].T
    return new_state, output


def kernel(**inputs):
    state = np.asarray(inputs["state"], dtype=np.float32)
    known = np.asarray(inputs["known_features"], dtype=np.float32)
    obs = np.asarray(inputs["observation"], dtype=np.float32)
    A = np.asarray(inputs["A_weight"], dtype=np.float32)
    in_maps = make_in_maps(state, known, obs, A)
    res = run_bass_kernel_spmd(get_nc(), in_maps, list(range(M)))
    return assemble(res.results)


# revision 32
# speedup vs baseline: 1.1459x; 1.1459x over previous
"""HCNN known-U cell on 8 Trainium2 NeuronCores (Bass/Tile).

Math (from the reference, simplified):
    output    = state[:, :16] - observation
    pre       = [observation, state[:, 16:], known_features]     # (B, 288)
    new_state = tanh(pre) @ A_weight.T                           # (B, 256)

Strategy: data-parallel over the batch (8192 rows/core). The host hands each
core *feature-major* (transposed) shards so the contraction dim (288 input
features) lands on SBUF partitions with no on-chip transposes, and every DMA
is a clean 2D strided copy with multi-KB lines. A_weight is replicated,
pre-transposed to (288, 256) so K-chunks of A^T serve as stationary lhsT.

The 288 contraction features are processed in a permuted order so that every
SBUF partition access starts 32-aligned (HW requirement):
    chunk0 (K=128): state feats  16:144
    chunk1 (K=112): state feats 144:256
    chunk2 (K= 48): known feats (32) then obs feats (16)  [obs at base 32]
A^T rows are permuted identically on the host; the K-sum is order-invariant.

Per core:
  inputs : state_t (256,8192), known_t (32,8192), obs_t (16,8192),
           a_t (288,256) permuted as above
  outputs: new_state_t (256,8192), out_t (16,8192)
"""

import sys

sys.path.insert(0, "/opt/trn_rl_repo")

import numpy as np

import concourse.bacc as bacc
import concourse.bass as bass
import concourse.tile as tile
from concourse import mybir
from concourse.bass_utils import run_bass_kernel_spmd

B, S, U, Y = 65536, 256, 32, 16
K = S + U  # 288 contraction features
M = 8  # cores
R = B // M  # 8192 batch rows per core
F = 1024  # batch columns per group
NSUB = 512  # matmul moving free dim (one PSUM bank of fp32)
FP = mybir.dt.float32
MM_DT = mybir.dt.float16  # matmul operand dtype: fp16 = full rate + FWL, ~3e-4 rel err

_nc_cache = {}


def _build(mm_dt):
    # Bacc (not raw Bass): its compile() pipeline splits multi-wait sync
    # conditions into event-semaphore instructions (TRN2 allows only one
    # sync wait per instruction).
    nc = bacc.Bacc("TRN2", target_bir_lowering=False, debug=False, num_devices=M)
    # state cols 16:256 (the only ones tanh needs) travel as fp16; cols 0:16
    # (only used for the exact output subtract) travel as fp32 in s0_t.
    st = nc.declare_dram_parameter("sm_t", [S - Y, R], mm_dt, isOutput=False)
    ko = nc.declare_dram_parameter("ko_t", [U + Y, R], FP, isOutput=False)
    s0t = nc.declare_dram_parameter("s0_t", [Y, R], FP, isOutput=False)
    at = nc.declare_dram_parameter("a_t", [128, 3 * S], mm_dt, isOutput=False)
    ns = nc.declare_dram_parameter("new_state_t", [S, R], mm_dt, isOutput=True)
    o2 = nc.declare_dram_parameter("out_t", [Y, R], FP, isOutput=True)

    Tanh = mybir.ActivationFunctionType.Tanh

    with tile.TileContext(nc) as tc:
        with (
            tc.tile_pool(name="const", bufs=1) as cpool,
            tc.tile_pool(name="ins", bufs=7) as ipool,
            tc.tile_pool(name="acts", bufs=6) as apool,
            tc.tile_pool(name="outs", bufs=6) as opool,
            tc.tile_pool(name="ps", bufs=3, space="PSUM") as pspool,
        ):
            # A^T packed on host into one (128, 768) block: cols 0:256 are
            # K-chunk0 (128 rows), 256:512 chunk1 (112 rows + pad), 512:768
            # chunk2 (48 rows + pad) -> a single DMA, fewer sync waits on the
            # first matmul.
            apk = cpool.tile([128, 3 * S], mm_dt, tag="apk")
            nc.sync.dma_start(apk[:], at[:, :])
            a0 = apk[:, 0:S]
            a1 = apk[0:112, S : 2 * S]
            a2 = apk[0:48, 2 * S : 3 * S]

            # Warm-up: ~20 dense dummy matmuls while the first DMAs stream,
            # so the PE HAM un-throttles (1.2 -> 2.4 GHz) before real work.
            wl = cpool.tile([128, 128], mm_dt, tag="wl")
            wr = cpool.tile([128, NSUB], mm_dt, tag="wr")
            nc.vector.memset(wl[:], 0.0)
            nc.vector.memset(wr[:], 0.0)
            wps = pspool.tile([128, NSUB], FP, tag="wps", name="wps", bufs=1)
            for _ in range(10):
                nc.tensor.matmul(wps[:], wl[:], wr[:], start=True, stop=True)

            for j in range(0, R, F):
                js = slice(j, j + F)
                # state feats 16:144 / 144:256 (fp16); known+obs packed fp32
                p0 = ipool.tile([128, F], mm_dt, tag="p0")
                p1 = ipool.tile([112, F], mm_dt, tag="p1")
                p2 = ipool.tile([48, F], FP, tag="p2")
                # state[:, :16] parked at base partition 32 so the subtract's
                # operands share a base partition (HW TensorTensor rule).
                s0 = ipool.tile([48, F], FP, tag="s0")
                nc.sync.dma_start(p0[:], st[0:128, js])
                nc.sync.dma_start(p1[:], st[128:240, js])
                nc.sync.dma_start(p2[:], ko[:, js])
                nc.sync.dma_start(s0[U : U + Y, :], s0t[:, js])

                # output = state[:, :16] - observation  (transposed layout)
                o2t = opool.tile([48, F], FP, tag="o2")
                nc.vector.tensor_sub(
                    o2t[U : U + Y, :], s0[U : U + Y, :], p2[U : U + Y, :]
                )
                nc.gpsimd.dma_start(o2[:, js], o2t[U : U + Y, :])

                # rstate^T = tanh(pre^T) with K features on partitions
                t0 = apool.tile([128, F], mm_dt, tag="t0")
                t1 = apool.tile([112, F], mm_dt, tag="t1")
                t2 = apool.tile([48, F], mm_dt, tag="t2")
                nc.scalar.activation(t0[:], p0[:], Tanh)
                nc.scalar.activation(t1[:], p1[:], Tanh)
                nc.scalar.activation(t2[:], p2[:], Tanh)

                # 2 filler matmuls per group: keep the PE HAM activity window
                # non-idle so the clock stays at 2.4 GHz through DMA stalls.
                nc.tensor.matmul(wps[0:64, 0:64], wl[:, 0:64], wr[:, 0:64], start=True, stop=True)

                n0 = opool.tile([128, F], mm_dt, tag="n0")
                n1 = opool.tile([128, F], mm_dt, tag="n1")
                # k-outer within each output half: the same 128x128 weight
                # block feeds all N-chunks back-to-back (one weight load per
                # chunk instead of one per matmul).
                for lo, ntile in ((0, n0), (128, n1)):
                    ps = pspool.tile([128, F], FP, tag="ps", name="ps")
                    for ki, (ak, tk) in enumerate(((a0, t0), (a1, t1), (a2, t2))):
                        for c in range(0, F, NSUB):
                            nc.tensor.matmul(
                                ps[:, c : c + NSUB],
                                ak[:, lo : lo + 128],
                                tk[:, c : c + NSUB],
                                start=(ki == 0),
                                stop=(ki == 2),
                            )
                    nc.vector.tensor_copy(ntile[:], ps[:])
                    nc.tensor.matmul(wps[0:64, 0:64], wl[:, 0:64], wr[:, 0:64], start=True, stop=True)
                nc.gpsimd.dma_start(ns[0:128, js], n0[:])
                nc.gpsimd.dma_start(ns[128:256, js], n1[:])
    nc.finalize()
    return nc


def get_nc():
    key = str(MM_DT)
    if key not in _nc_cache:
        _nc_cache[key] = _build(MM_DT)
    return _nc_cache[key]


def _np_mm_dtype():
    if MM_DT == mybir.dt.bfloat16:
        import ml_dtypes

        return ml_dtypes.bfloat16
    if MM_DT == mybir.dt.float16:
        return np.float16
    return np.float32


def make_in_maps(state, known, obs, A):
    at = np.ascontiguousarray(A.T)  # (288, 256); row i = input feature i
    # Pack permuted K-chunks side by side into one (128, 768) block (see
    # module docstring); pad rows are zero and multiply into nothing.
    a_perm = np.zeros((128, 3 * S), dtype=np.float32)
    a_perm[0:128, 0:S] = at[16:144]
    a_perm[0:112, S : 2 * S] = at[144:256]
    a_perm[0:32, 2 * S : 3 * S] = at[256:288]
    a_perm[32:48, 2 * S : 3 * S] = at[0:16]
    a_perm = a_perm.astype(_np_mm_dtype())
    in_maps = []
    for i in range(M):
        sl = slice(i * R, (i + 1) * R)
        in_maps.append(
            {
                "sm_t": np.ascontiguousarray(state[sl, Y:].T).astype(_np_mm_dtype()),
                "ko_t": np.ascontiguousarray(
                    np.concatenate([known[sl], obs[sl]], axis=1).T
                ),
                "s0_t": np.ascontiguousarray(state[sl, :Y].T),
                "a_t": a_perm,
            }
        )
    return in_maps


def assemble(results):
    new_state = np.empty((B, S), np.float32)
    output = np.empty((B, Y), np.float32)
    for i in range(M):
        sl = slice(i * R, (i + 1) * R)
        new_state[sl] = results[i]["new_state_t"].T.astype(np.float32)
        output[sl] = results[i]["out_t"].T
    return new_state, output


def kernel(**inputs):
    state = np.asarray(inputs["state"], dtype=np.float32)
    known = np.asarray(inputs["known_features"], dtype=np.float32)
    obs = np.asarray(inputs["observation"], dtype=np.float32)
    A = np.asarray(inputs["A_weight"], dtype=np.float32)
    in_maps = make_in_maps(state, known, obs, A)
    res = run_bass_kernel_spmd(get_nc(), in_maps, list(range(M)))
    return assemble(res.results)


# revision 33
# speedup vs baseline: 1.2587x; 1.0984x over previous
"""HCNN known-U cell on 8 Trainium2 NeuronCores (Bass/Tile).

Math (from the reference, simplified):
    output    = state[:, :16] - observation
    pre       = [observation, state[:, 16:], known_features]     # (B, 288)
    new_state = tanh(pre) @ A_weight.T                           # (B, 256)

Strategy: data-parallel over the batch (8192 rows/core). The host hands each
core *feature-major* (transposed) shards so the contraction dim (288 input
features) lands on SBUF partitions with no on-chip transposes, and every DMA
is a clean 2D strided copy with multi-KB lines. A_weight is replicated,
pre-transposed/permuted so K-chunks of A^T serve as stationary lhsT.

The 288 contraction features are processed in a permuted order so that every
SBUF partition access starts 32-aligned (HW requirement):
    chunk0 (K=128): state feats  16:144
    chunk1 (K=112): state feats 144:256
    chunk2 (K= 48): known feats (32) then obs feats (16)  [obs at base 32]
A^T rows are permuted identically on the host; the K-sum is order-invariant.

Precision: matmul operands (tanh outputs, A) and the state/new_state DMA
transport are fp16 (~3e-4 rel err); the observation/known inputs and the
`output` subtract stay exact fp32. PSUM accumulation is fp32.

Perf notes (HW-profiled): warm-up matmuls un-throttle the PE HAM clock gate
before real work; tiny per-group filler matmuls keep the activity window
non-idle so the PE stays at 2.4 GHz; input DMAs issue on the sync ring,
output DMAs on the gpsimd ring; 2-bank PSUM tiles take one fp32->fp16 cast
each. ~54 us/core vs ~30 us DMA roofline + ~16 us fixed framework overhead.

Per core:
  inputs : sm_t (240,8192) fp16 = state[:,16:]^T, ko_t (48,8192) fp32 =
           [known; obs]^T, s0_t (16,8192) fp32 = state[:,:16]^T,
           a_t (128,768) fp16 packed/permuted A^T chunks
  outputs: new_state_t (256,8192) fp16, out_t (16,8192) fp32
"""

import sys

sys.path.insert(0, "/opt/trn_rl_repo")

import numpy as np

import concourse.bacc as bacc
import concourse.tile as tile
from concourse import mybir
from concourse.bass_utils import run_bass_kernel_spmd

B, S, U, Y = 65536, 256, 32, 16
K = S + U  # 288 contraction features
M = 8  # cores
R = B // M  # 8192 batch rows per core
F = 1024  # batch columns per group
NSUB = 512  # matmul moving free dim (one PSUM bank of fp32)
FP = mybir.dt.float32
MM_DT = mybir.dt.float16  # matmul operand dtype: fp16 = full rate + FWL, ~3e-4 rel err

_nc_cache = {}


def _build(mm_dt):
    # Bacc (not raw Bass): its compile() pipeline splits multi-wait sync
    # conditions into event-semaphore instructions (TRN2 allows only one
    # sync wait per instruction).
    nc = bacc.Bacc("TRN2", target_bir_lowering=False, debug=False, num_devices=M)
    # state cols 16:256 (the only ones tanh needs) travel as fp16; cols 0:16
    # (only used for the exact output subtract) travel as fp32 in s0_t.
    st = nc.declare_dram_parameter("sm_t", [S - Y, R], mm_dt, isOutput=False)
    ko = nc.declare_dram_parameter("ko_t", [U + Y, R], FP, isOutput=False)
    s0t = nc.declare_dram_parameter("s0_t", [Y, R], FP, isOutput=False)
    at = nc.declare_dram_parameter("a_t", [128, 3 * S], mm_dt, isOutput=False)
    ns = nc.declare_dram_parameter("new_state_t", [S, R], mm_dt, isOutput=True)
    o2 = nc.declare_dram_parameter("out_t", [Y, R], FP, isOutput=True)

    Tanh = mybir.ActivationFunctionType.Tanh

    with tile.TileContext(nc) as tc:
        with (
            tc.tile_pool(name="const", bufs=1) as cpool,
            tc.tile_pool(name="ins", bufs=7) as ipool,
            tc.tile_pool(name="acts", bufs=6) as apool,
            tc.tile_pool(name="outs", bufs=6) as opool,
            tc.tile_pool(name="ps", bufs=3, space="PSUM") as pspool,
        ):
            # A^T packed on host into one (128, 768) block: cols 0:256 are
            # K-chunk0 (128 rows), 256:512 chunk1 (112 rows + pad), 512:768
            # chunk2 (48 rows + pad) -> a single DMA, fewer sync waits on the
            # first matmul.
            apk = cpool.tile([128, 3 * S], mm_dt, tag="apk")
            nc.sync.dma_start(apk[:], at[:, :])
            a0 = apk[:, 0:S]
            a1 = apk[0:112, S : 2 * S]
            a2 = apk[0:48, 2 * S : 3 * S]

            # Warm-up: ~20 dense dummy matmuls while the first DMAs stream,
            # so the PE HAM un-throttles (1.2 -> 2.4 GHz) before real work.
            wl = cpool.tile([128, 128], mm_dt, tag="wl")
            wr = cpool.tile([128, NSUB], mm_dt, tag="wr")
            nc.vector.memset(wl[:], 0.0)
            nc.vector.memset(wr[:], 0.0)
            wps = pspool.tile([128, NSUB], FP, tag="wps", name="wps", bufs=1)
            for _ in range(10):
                nc.tensor.matmul(wps[:], wl[:], wr[:], start=True, stop=True)

            for j in range(0, R, F):
                js = slice(j, j + F)
                # state feats 16:144 / 144:256 (fp16); known+obs packed fp32
                p0 = ipool.tile([128, F], mm_dt, tag="p0")
                p1 = ipool.tile([112, F], mm_dt, tag="p1")
                p2 = ipool.tile([48, F], FP, tag="p2")
                # state[:, :16] parked at base partition 32 so the subtract's
                # operands share a base partition (HW TensorTensor rule).
                s0 = ipool.tile([48, F], FP, tag="s0")
                nc.sync.dma_start(p0[:], st[0:128, js])
                nc.sync.dma_start(p1[:], st[128:240, js])
                nc.sync.dma_start(p2[:], ko[:, js])
                nc.sync.dma_start(s0[U : U + Y, :], s0t[:, js])

                # output = state[:, :16] - observation  (transposed layout)
                o2t = opool.tile([48, F], FP, tag="o2")
                nc.vector.tensor_sub(
                    o2t[U : U + Y, :], s0[U : U + Y, :], p2[U : U + Y, :]
                )
                nc.gpsimd.dma_start(o2[:, js], o2t[U : U + Y, :])

                # rstate^T = tanh(pre^T) with K features on partitions
                t0 = apool.tile([128, F], mm_dt, tag="t0")
                t1 = apool.tile([112, F], mm_dt, tag="t1")
                t2 = apool.tile([48, F], mm_dt, tag="t2")
                nc.scalar.activation(t0[:], p0[:], Tanh)
                nc.scalar.activation(t1[:], p1[:], Tanh)
                nc.scalar.activation(t2[:], p2[:], Tanh)

                # 2 filler matmuls per group: keep the PE HAM activity window
                # non-idle so the clock stays at 2.4 GHz through DMA stalls.
                nc.tensor.matmul(wps[0:64, 0:64], wl[:, 0:64], wr[:, 0:64], start=True, stop=True)

                n0 = opool.tile([128, F], mm_dt, tag="n0")
                n1 = opool.tile([128, F], mm_dt, tag="n1")
                # k-outer within each output half: the same 128x128 weight
                # block feeds all N-chunks back-to-back (one weight load per
                # chunk instead of one per matmul).
                for lo, ntile in ((0, n0), (128, n1)):
                    ps = pspool.tile([128, F], FP, tag="ps", name="ps")
                    for ki, (ak, tk) in enumerate(((a0, t0), (a1, t1), (a2, t2))):
                        for c in range(0, F, NSUB):
                            nc.tensor.matmul(
                                ps[:, c : c + NSUB],
                                ak[:, lo : lo + 128],
                                tk[:, c : c + NSUB],
                                start=(ki == 0),
                                stop=(ki == 2),
                            )
                    nc.vector.tensor_copy(ntile[:], ps[:])
                    nc.tensor.matmul(wps[0:64, 0:64], wl[:, 0:64], wr[:, 0:64], start=True, stop=True)
                nc.gpsimd.dma_start(ns[0:128, js], n0[:])
                nc.gpsimd.dma_start(ns[128:256, js], n1[:])
    nc.finalize()
    return nc


def get_nc():
    key = str(MM_DT)
    if key not in _nc_cache:
        _nc_cache[key] = _build(MM_DT)
    return _nc_cache[key]


def _np_mm_dtype():
    if MM_DT == mybir.dt.bfloat16:
        import ml_dtypes

        return ml_dtypes.bfloat16
    if MM_DT == mybir.dt.float16:
        return np.float16
    return np.float32


def make_in_maps(state, known, obs, A):
    at = np.ascontiguousarray(A.T)  # (288, 256); row i = input feature i
    # Pack permuted K-chunks side by side into one (128, 768) block (see
    # module docstring); pad rows are zero and multiply into nothing.
    a_perm = np.zeros((128, 3 * S), dtype=np.float32)
    a_perm[0:128, 0:S] = at[16:144]
    a_perm[0:112, S : 2 * S] = at[144:256]
    a_perm[0:32, 2 * S : 3 * S] = at[256:288]
    a_perm[32:48, 2 * S : 3 * S] = at[0:16]
    a_perm = a_perm.astype(_np_mm_dtype())
    in_maps = []
    for i in range(M):
        sl = slice(i * R, (i + 1) * R)
        in_maps.append(
            {
                "sm_t": np.ascontiguousarray(state[sl, Y:].T).astype(_np_mm_dtype()),
                "ko_t": np.ascontiguousarray(
                    np.concatenate([known[sl], obs[sl]], axis=1).T
                ),
                "s0_t": np.ascontiguousarray(state[sl, :Y].T),
                "a_t": a_perm,
            }
        )
    return in_maps


def assemble(results):
    new_state = np.empty((B, S), np.float32)
    output = np.empty((B, Y), np.float32)
    for i in range(M):
        sl = slice(i * R, (i + 1) * R)
        new_state[sl] = results[i]["new_state_t"].T.astype(np.float32)
        output[sl] = results[i]["out_t"].T
    return new_state, output


def kernel(**inputs):
    state = np.asarray(inputs["state"], dtype=np.float32)
    known = np.asarray(inputs["known_features"], dtype=np.float32)
    obs = np.asarray(inputs["observation"], dtype=np.float32)
    A = np.asarray(inputs["A_weight"], dtype=np.float32)
    in_maps = make_in_maps(state, known, obs, A)
    res = run_bass_kernel_spmd(get_nc(), in_maps, list(range(M)))
    return assemble(res.results)


# revision 34
# speedup vs baseline: 1.2602x; 1.0012x over previous
"""HCNN known-U cell on 8 Trainium2 NeuronCores (Bass/Tile).

Math (from the reference, simplified):
    output    = state[:, :16] - observation
    pre       = [observation, state[:, 16:], known_features]     # (B, 288)
    new_state = tanh(pre) @ A_weight.T                           # (B, 256)

Strategy: data-parallel over the batch (8192 rows/core). The host hands each
core *feature-major* (transposed) shards so the contraction dim (288 input
features) lands on SBUF partitions with no on-chip transposes, and every DMA
is a clean 2D strided copy with multi-KB lines. A_weight is replicated,
pre-transposed/permuted so K-chunks of A^T serve as stationary lhsT.

The 288 contraction features are processed in a permuted order so that every
SBUF partition access starts 32-aligned (HW requirement):
    chunk0 (K=128): state feats  16:144
    chunk1 (K=112): state feats 144:256
    chunk2 (K= 48): known feats (32) then obs feats (16)  [obs at base 32]
A^T rows are permuted identically on the host; the K-sum is order-invariant.

Precision: matmul operands (tanh outputs, A) and the state/new_state DMA
transport are fp16 (~3e-4 rel err); the observation/known inputs and the
`output` subtract stay exact fp32. PSUM accumulation is fp32.

Perf notes (HW-profiled): warm-up matmuls un-throttle the PE HAM clock gate
before real work; tiny per-group filler matmuls keep the activity window
non-idle so the PE stays at 2.4 GHz; input DMAs issue on the sync ring,
output DMAs on the gpsimd ring; 2-bank PSUM tiles take one fp32->fp16 cast
each. ~54 us/core vs ~30 us DMA roofline + ~16 us fixed framework overhead.

Per core:
  inputs : sm_t (240,8192) fp16 = state[:,16:]^T, ko_t (48,8192) fp32 =
           [known; obs]^T, s0_t (16,8192) fp32 = state[:,:16]^T,
           a_t (128,768) fp16 packed/permuted A^T chunks
  outputs: new_state_t (256,8192) fp16, out_t (16,8192) fp32
"""

import sys

sys.path.insert(0, "/opt/trn_rl_repo")

import numpy as np

import concourse.bacc as bacc
import concourse.tile as tile
from concourse import mybir
from concourse.bass_utils import run_bass_kernel_spmd

B, S, U, Y = 65536, 256, 32, 16
K = S + U  # 288 contraction features
M = 8  # cores
R = B // M  # 8192 batch rows per core
F = 1024  # batch columns per group
NSUB = 512  # matmul moving free dim (one PSUM bank of fp32)
FP = mybir.dt.float32
MM_DT = mybir.dt.float16  # matmul operand dtype: fp16 = full rate + FWL, ~3e-4 rel err

_nc_cache = {}


def _build(mm_dt):
    # Bacc (not raw Bass): its compile() pipeline splits multi-wait sync
    # conditions into event-semaphore instructions (TRN2 allows only one
    # sync wait per instruction).
    nc = bacc.Bacc("TRN2", target_bir_lowering=False, debug=False, num_devices=M)
    # state cols 16:256 (the only ones tanh needs) travel as fp16; cols 0:16
    # (only used for the exact output subtract) travel as fp32 in s0_t.
    st = nc.declare_dram_parameter("sm_t", [S - Y, R], mm_dt, isOutput=False)
    ko = nc.declare_dram_parameter("ko_t", [U + Y, R], FP, isOutput=False)
    s0t = nc.declare_dram_parameter("s0_t", [Y, R], FP, isOutput=False)
    at = nc.declare_dram_parameter("a_t", [128, 3 * S], mm_dt, isOutput=False)
    ns = nc.declare_dram_parameter("new_state_t", [S, R], mm_dt, isOutput=True)
    o2 = nc.declare_dram_parameter("out_t", [Y, R], FP, isOutput=True)

    Tanh = mybir.ActivationFunctionType.Tanh

    with tile.TileContext(nc) as tc:
        with (
            tc.tile_pool(name="const", bufs=1) as cpool,
            tc.tile_pool(name="ins", bufs=7) as ipool,
            tc.tile_pool(name="acts", bufs=7) as apool,
            tc.tile_pool(name="outs", bufs=7) as opool,
            tc.tile_pool(name="ps", bufs=3, space="PSUM") as pspool,
        ):
            # A^T packed on host into one (128, 768) block: cols 0:256 are
            # K-chunk0 (128 rows), 256:512 chunk1 (112 rows + pad), 512:768
            # chunk2 (48 rows + pad) -> a single DMA, fewer sync waits on the
            # first matmul.
            apk = cpool.tile([128, 3 * S], mm_dt, tag="apk")
            nc.sync.dma_start(apk[:], at[:, :])
            a0 = apk[:, 0:S]
            a1 = apk[0:112, S : 2 * S]
            a2 = apk[0:48, 2 * S : 3 * S]

            # Warm-up: ~20 dense dummy matmuls while the first DMAs stream,
            # so the PE HAM un-throttles (1.2 -> 2.4 GHz) before real work.
            wl = cpool.tile([128, 128], mm_dt, tag="wl")
            wr = cpool.tile([128, NSUB], mm_dt, tag="wr")
            nc.vector.memset(wl[:], 0.0)
            nc.vector.memset(wr[:], 0.0)
            wps = pspool.tile([128, NSUB], FP, tag="wps", name="wps", bufs=1)
            for _ in range(10):
                nc.tensor.matmul(wps[:], wl[:], wr[:], start=True, stop=True)

            for j in range(0, R, F):
                js = slice(j, j + F)
                # state feats 16:144 / 144:256 (fp16); known+obs packed fp32
                p0 = ipool.tile([128, F], mm_dt, tag="p0")
                p1 = ipool.tile([112, F], mm_dt, tag="p1")
                p2 = ipool.tile([48, F], FP, tag="p2")
                # state[:, :16] parked at base partition 32 so the subtract's
                # operands share a base partition (HW TensorTensor rule).
                s0 = ipool.tile([48, F], FP, tag="s0")
                nc.sync.dma_start(p0[:], st[0:128, js])
                nc.sync.dma_start(p1[:], st[128:240, js])
                nc.sync.dma_start(p2[:], ko[:, js])
                nc.sync.dma_start(s0[U : U + Y, :], s0t[:, js])

                # output = state[:, :16] - observation  (transposed layout)
                o2t = opool.tile([48, F], FP, tag="o2")
                nc.vector.tensor_sub(
                    o2t[U : U + Y, :], s0[U : U + Y, :], p2[U : U + Y, :]
                )
                nc.gpsimd.dma_start(o2[:, js], o2t[U : U + Y, :])

                # rstate^T = tanh(pre^T) with K features on partitions
                t0 = apool.tile([128, F], mm_dt, tag="t0")
                t1 = apool.tile([112, F], mm_dt, tag="t1")
                t2 = apool.tile([48, F], mm_dt, tag="t2")
                nc.scalar.activation(t0[:], p0[:], Tanh)
                nc.scalar.activation(t1[:], p1[:], Tanh)
                nc.scalar.activation(t2[:], p2[:], Tanh)

                # 2 filler matmuls per group: keep the PE HAM activity window
                # non-idle so the clock stays at 2.4 GHz through DMA stalls.
                nc.tensor.matmul(wps[0:64, 0:64], wl[:, 0:64], wr[:, 0:64], start=True, stop=True)

                n0 = opool.tile([128, F], mm_dt, tag="n0")
                n1 = opool.tile([128, F], mm_dt, tag="n1")
                # k-outer within each output half: the same 128x128 weight
                # block feeds all N-chunks back-to-back (one weight load per
                # chunk instead of one per matmul).
                for lo, ntile in ((0, n0), (128, n1)):
                    ps = pspool.tile([128, F], FP, tag="ps", name="ps")
                    for ki, (ak, tk) in enumerate(((a0, t0), (a1, t1), (a2, t2))):
                        for c in range(0, F, NSUB):
                            nc.tensor.matmul(
                                ps[:, c : c + NSUB],
                                ak[:, lo : lo + 128],
                                tk[:, c : c + NSUB],
                                start=(ki == 0),
                                stop=(ki == 2),
                            )
                    nc.vector.tensor_copy(ntile[:], ps[:])
                    nc.tensor.matmul(wps[0:64, 0:64], wl[:, 0:64], wr[:, 0:64], start=True, stop=True)
                nc.gpsimd.dma_start(ns[0:128, js], n0[:])
                nc.gpsimd.dma_start(ns[128:256, js], n1[:])
    nc.finalize()
    return nc


def get_nc():
    key = str(MM_DT)
    if key not in _nc_cache:
        _nc_cache[key] = _build(MM_DT)
    return _nc_cache[key]


def _np_mm_dtype():
    if MM_DT == mybir.dt.bfloat16:
        import ml_dtypes

        return ml_dtypes.bfloat16
    if MM_DT == mybir.dt.float16:
        return np.float16
    return np.float32


def make_in_maps(state, known, obs, A):
    at = np.ascontiguousarray(A.T)  # (288, 256); row i = input feature i
    # Pack permuted K-chunks side by side into one (128, 768) block (see
    # module docstring); pad rows are zero and multiply into nothing.
    a_perm = np.zeros((128, 3 * S), dtype=np.float32)
    a_perm[0:128, 0:S] = at[16:144]
    a_perm[0:112, S : 2 * S] = at[144:256]
    a_perm[0:32, 2 * S : 3 * S] = at[256:288]
    a_perm[32:48, 2 * S : 3 * S] = at[0:16]
    a_perm = a_perm.astype(_np_mm_dtype())
    in_maps = []
    for i in range(M):
        sl = slice(i * R, (i + 1) * R)
        in_maps.append(
            {
                "sm_t": np.ascontiguousarray(state[sl, Y:].T).astype(_np_mm_dtype()),
                "ko_t": np.ascontiguousarray(
                    np.concatenate([known[sl], obs[sl]], axis=1).T
                ),
                "s0_t": np.ascontiguousarray(state[sl, :Y].T),
                "a_t": a_perm,
            }
        )
    return in_maps


def assemble(results):
    new_state = np.empty((B, S), np.float32)
    output = np.empty((B, Y), np.float32)
    for i in range(M):
        sl = slice(i * R, (i + 1) * R)
        new_state[sl] = results[i]["new_state_t"].T.astype(np.float32)
        output[sl] = results[i]["out_t"].T
    return new_state, output


def kernel(**inputs):
    state = np.asarray(inputs["state"], dtype=np.float32)
    known = np.asarray(inputs["known_features"], dtype=np.float32)
    obs = np.asarray(inputs["observation"], dtype=np.float32)
    A = np.asarray(inputs["A_weight"], dtype=np.float32)
    in_maps = make_in_maps(state, known, obs, A)
    res = run_bass_kernel_spmd(get_nc(), in_maps, list(range(M)))
    return assemble(res.results)
